# revision 20
# baseline (speedup 1.0000x reference)
"""5G LDPC BG1 encoder (k=8000, n=16000, r=0.5, Z=384) on 8 Trainium2 cores.

Strategy: data parallelism over the batch (2048 -> 8 cores x 256 rows) with
4-way nibble packing: 4 batch rows share one uint16 SBUF lane (row t*64+p ->
nibble t of partition p), so every engine op processes 4 codewords at once
and DMA moves 4x fewer bytes.  GF(2) addition is nibble-wise bitwise XOR
(DVE-only on TRN2); nibbles stay in {0,1} so the host recovers bits with a
shift-and-mask.  Circulant shifts use a halo copy of every 384-col block,
loaded by a second DMA pass straight from DRAM.  Independent XOR-chain steps
from two different rows are fused into one [P,2,384] DVE instruction via
hand-built access patterns (arbitrary stride between the two rows), halving
DVE instruction count.  The rate-matching interleaver (out[:,4j+i] =
c_short[i*4000+j]) runs as stride-4 packed copies split between Activation
and GpSimd: u/pa-sourced output phases are emitted early, pb-sourced spans
per chunk as parity rows complete.  Host work is layout-only: pack nibbles
in, shift-and-mask out.
"""
import numpy as np
from contextlib import ExitStack

Z = 384
KB = 22
K = 8000
N = 16000
K_LDPC = KB * Z          # 8448
NBPS = 4
NQ = N // NBPS           # 4000
PB_BLOCKS = 19           # only pb[0:7232] survives rate matching

B_TOTAL = 2048
N_CORES = 8
B_CORE = B_TOTAL // N_CORES   # 256
P = 64                        # partitions per core
PACK = 4                      # batch rows packed per uint16 lane (nibbles)
NCHUNK = 8                    # output column chunks of 2000

_CACHE = {}


def _base_entries(rows, cols):
    """Recover (base_row, base_col, shift) triplets from lifted index lists."""
    rows = np.asarray(rows, np.int64)
    cols = np.asarray(cols, np.int64)
    m = (rows % Z) == 0
    br = (rows[m] // Z).astype(int)
    bc = (cols[m] // Z).astype(int)
    sh = (cols[m] % Z).astype(int)
    return list(zip(br.tolist(), bc.tolist(), sh.tolist()))


def _group(entries, n_blocks, drop_bc=()):
    g = [[] for _ in range(n_blocks)]
    for br, bc, s in entries:
        if bc in drop_bc or br >= n_blocks:
            continue
        g[br].append((bc, s))
    return g


def _ilv_copies(chunk, nchunk=NCHUNK):
    """Interleaver copy specs for output chunk (cols [chunk*cw, +cw)):
    (tile, blk0, off, nblk, ln, dst_start_within_chunk).

    c_short = u_bits[768:8000] ++ pa[0:1536] ++ pb[0:7232], and
    out[:, 4j+i] = c_short[i*4000 + j]; chunk c covers j in
    [c*(NQ/nchunk), (c+1)*(NQ/nchunk)).
    """
    spans = ([("u", b, 0, Z) for b in range(2, 20)] + [("u", 20, 0, 320)]
             + [("pa", b, 0, Z) for b in range(4)]
             + [("pb", b, 0, Z) for b in range(18)] + [("pb", 18, 0, 320)])
    jlo, jhi = chunk * (NQ // nchunk), (chunk + 1) * (NQ // nchunk)
    out = []
    for i in range(NBPS):
        # phase i reads c_short[i*NQ + j] for j in [jlo, jhi); dst offsets
        # are absolute within the [P, N] output tile.
        glo, ghi = i * NQ + jlo, i * NQ + jhi
        g = 0
        pieces = []
        for tname, blk, off, ln in spans:
            a, b = max(g, glo), min(g + ln, ghi)
            if a < b:
                pieces.append((tname, blk, off + a - g, b - a,
                               4 * (a - glo) + i + 4 * jlo))
            g += ln
        merged = []
        for pc in pieces:
            tname, blk, off, ln, ds = pc
            if merged and off == 0 and ln == Z:
                mt, mb_, mo, mn, ml, mds = merged[-1]
                if mt == tname and mo == 0 and ml == Z and mb_ + mn == blk:
                    merged[-1] = (mt, mb_, mo, mn + 1, ml, mds)
                    continue
            merged.append((tname, blk, off, 1, ln, ds))
        out.extend(merged)
    return out


def _build_program(gA, gC1, gC2):
    import concourse.tile as tile
    from concourse import bacc, mybir
    from concourse.alu_op_type import AluOpType
    import bass_rust

    u16 = mybir.dt.uint16
    XOR = AluOpType.bitwise_xor
    VecI64Pair = bass_rust.VecI64Pair

    nc = bacc.Bacc("TRN2", target_bir_lowering=False, debug=False)
    u_dram = nc.dram_tensor("u", [P, K], u16, kind="ExternalInput").ap()
    o_dram = nc.dram_tensor("out", [P, N], u16, kind="ExternalOutput").ap()

    def pair_view(flat_ap, addr_a, addr_b, ln=Z):
        """[P, 2, ln] view of a flat [P, M] tile AP at two free offsets."""
        v = flat_ap[:, addr_a:addr_a + 1]
        w = v.copy()
        pstride = v.ap.to_list()[0]
        w.ap = VecI64Pair([pstride, [addr_b - addr_a, 2], [1, ln]])
        return w

    with tile.TileContext(nc) as tc, ExitStack() as ctx:
        pin = ctx.enter_context(tc.tile_pool(name="pin", bufs=1))
        pw = ctx.enter_context(tc.tile_pool(name="pw", bufs=1))
        pout = ctx.enter_context(tc.tile_pool(name="pout", bufs=1))

        # u_dup[p, bc*768 + 0:384] = info block bc; [.. 384:768] = halo copy
        # (blocks 0..20; block 21 is all-filler and dropped from the graph).
        # DMA in per block-group, main+halo interleaved, so XOR waves that
        # touch low blocks can start while later groups are still in flight.
        u_dup = pin.tile([P, 21 * 2 * Z], u16, tag="u_dup")
        u3 = u_dup.rearrange("p (a b) -> p a b", b=2 * Z)
        nc.gpsimd.memset(u3[:, 20, 320:Z], 0)
        nc.gpsimd.memset(u3[:, 20, Z + 320:2 * Z], 0)
        for lo, hi in ((0, 2), (2, 6), (6, 12), (12, 20)):
            src = u_dram[:, lo * Z:hi * Z].rearrange("p (a b) -> p a b", b=Z)
            nc.sync.dma_start(u3[:, lo:hi, 0:Z], src)
            nc.sync.dma_start(u3[:, lo:hi, Z:2 * Z], src)
        nc.sync.dma_start(u3[:, 20, 0:320], u_dram[:, 7680:8000])
        nc.sync.dma_start(u3[:, 20, Z:Z + 320], u_dram[:, 7680:8000])

        # work tile: au rows 0..3 then pb rows 0..18, each Z wide (flat)
        work = pw.tile([P, (4 + PB_BLOCKS) * Z], u16, tag="work")
        pa = pw.tile([P, 4 * 2 * Z], u16, tag="pa")
        pa3 = pa.rearrange("p (a b) -> p a b", b=2 * Z)

        def uaddr(bc, s):
            return bc * 2 * Z + s

        def paaddr(bc, s):
            return bc * 2 * Z + s

        def au_a(br):
            return br * Z

        def pb_a(r):
            return (4 + r) * Z

        # ---------- wave scheduler: pair-fuse independent chain steps ------
        # rows: au rows then pb rows.  Each row = (dst_addr, [steps]);
        # step = ('first', a1, a2) | ('accu', a) | ('accpa', a).
        emitted = {"n": 0}

        def emit_steps(s1, s2):
            """Emit one DVE instruction covering one or two chain steps."""
            if s2 is None:
                dst_a, st = s1
                if st[0] == "first":
                    nc.vector.tensor_tensor(
                        work[:, dst_a:dst_a + Z],
                        u_dup[:, st[1]:st[1] + Z],
                        u_dup[:, st[2]:st[2] + Z], op=XOR)
                else:
                    src = u_dup if st[0] == "accu" else pa
                    nc.vector.tensor_tensor(
                        work[:, dst_a:dst_a + Z],
                        work[:, dst_a:dst_a + Z],
                        src[:, st[1]:st[1] + Z], op=XOR)
                return
            (da, sta), (db, stb) = s1, s2
            dst = pair_view(work, da, db)
            if sta[0] == "first":
                nc.vector.tensor_tensor(
                    dst, pair_view(u_dup, sta[1], stb[1]),
                    pair_view(u_dup, sta[2], stb[2]), op=XOR)
            else:
                src = u_dup if sta[0] == "accu" else pa
                nc.vector.tensor_tensor(
                    dst, pair_view(work, da, db),
                    pair_view(src, sta[1], stb[1]), op=XOR)

        def run_waves(rows, on_row_done=None):
            """rows: list of (row_key, dst_addr, state) with state =
            {'first': (a1,a2)|None, 'accu': [a..], 'accpa': [a..]}.
            Two-server greedy by list order: each wave advances the foremost
            unfinished row, fused with the next row that can offer a
            same-class step ('first' must precede accs within a row; accu
            and accpa commute).  Completion order == list order."""
            def avail(st):
                if st["first"] is not None:
                    return ("first",)
                cl = []
                if st["accu"]:
                    cl.append("accu")
                if st["accpa"]:
                    cl.append("accpa")
                return tuple(cl)

            def take(st, cls):
                if cls == "first":
                    a1, a2 = st["first"]
                    st["first"] = None
                    return ("first", a1, a2)
                if cls == "accu":
                    return ("accu", st["accu"].pop(0))
                return ("accpa", st["accpa"].pop(0))

            def row_done(k):
                if not avail(rows[k][2]) and on_row_done:
                    on_row_done(rows[k][0])

            while True:
                act = [i for i, r in enumerate(rows) if avail(r[2])]
                if not act:
                    break
                i = act[0]
                ci = avail(rows[i][2])
                pick = None
                for j in act[1:]:
                    shared = [c for c in ci if c in avail(rows[j][2])]
                    if shared:
                        pick = (j, shared[0])
                        break
                if pick is None:
                    # prefer draining accu first (pa may not be ready early)
                    cls = ci[0]
                    emit_steps((rows[i][1], take(rows[i][2], cls)), None)
                    row_done(i)
                else:
                    j, cls = pick
                    emit_steps((rows[i][1], take(rows[i][2], cls)),
                               (rows[j][1], take(rows[j][2], cls)))
                    row_done(i)
                    row_done(j)

        def mk_state(c1_terms, c2_terms):
            us = sorted(uaddr(bc, s) for bc, s in c1_terms)
            pas = sorted(paaddr(bc, s) for bc, s in c2_terms)
            assert len(us) >= 2
            return {"first": (us[0], us[1]), "accu": us[2:], "accpa": pas}

        # ---- au = A @ u ----
        au_rows = [(("au", br), au_a(br), mk_state(gA[br], []))
                   for br in range(4)]
        run_waves(au_rows)

        # ---- pa = B_inv @ au = cumulative XOR, written to main AND halo
        # half in one dual-write op each (no separate halo copy) ----
        nc.vector.tensor_copy(pair_view(pa, 0, Z),
                              pair_view(work, au_a(0), au_a(0)))
        for i in range(1, 4):
            nc.vector.tensor_tensor(
                pair_view(pa, i * 2 * Z, i * 2 * Z + Z),
                pair_view(pa, (i - 1) * 2 * Z, (i - 1) * 2 * Z),
                pair_view(work, au_a(i), au_a(i)), op=XOR)

        # ---- pb rows 16..18: integer add-accumulate chains on the software
        # DGE (nibble counts <= 15; host parity-extracts).  Costs ~1us of
        # GpSimd time per hop but runs off the DVE critical path.  WAW on
        # the destination serializes each chain; u-hops first so the queue
        # never stalls waiting for pa. ----
        dma_rows = tuple(r for r in (16, 17, 18)
                         if len(gC1[r]) + len(gC2[r]) <= 15)
        hops_u, hops_pa = [], []
        for r in dma_rows:
            us = sorted(uaddr(bc, s) for bc, s in gC1[r])
            pas = sorted(paaddr(bc, s) for bc, s in gC2[r])
            hops_u.append((r, us))
            hops_pa.append((r, pas))
        for r, us in hops_u:
            dst = work[:, pb_a(r):pb_a(r) + Z]
            nc.gpsimd.dma_start(dst, u_dup[:, us[0]:us[0] + Z])
            for a in us[1:]:
                nc.gpsimd.dma_start(dst, u_dup[:, a:a + Z],
                                    accum_op=AluOpType.add)
        for r, pas in hops_pa:
            dst = work[:, pb_a(r):pb_a(r) + Z]
            for a in pas:
                nc.gpsimd.dma_start(dst, pa[:, a:a + Z],
                                    accum_op=AluOpType.add)

        # ---- interleave: balance Act vs Pool by cost (Pool pre-loaded
        # with the SWDGE hop cost above) ----
        of = pout.tile([P, N], u16, tag="of")
        cw = N // NCHUNK
        nhops = sum(len(us) for _, us in hops_u) + sum(
            len(p) for _, p in hops_pa)
        ebusy = {"act": 0.0, "pool": 1020.0 * nhops}

        def ilv_op(dst, src, ln, eng=None):
            if eng == "dve":
                nc.vector.tensor_copy(dst, src)
                return
            ca = 0.833 * ln + 185          # Activation: fast, big fixed cost
            cp = 1.389 * ln + 60           # GpSimd: slower, no fixed cost
            if ebusy["act"] + ca <= ebusy["pool"] + cp:
                ebusy["act"] += ca
                nc.scalar.copy(dst, src)
            else:
                ebusy["pool"] += cp
                nc.gpsimd.tensor_copy(dst, src)

        def emit_ilv(c, tname_sel, nchunk=NCHUNK, eng=None):
            for tname, blk0, off, nblk, ln, ds in _ilv_copies(c, nchunk):
                if tname != tname_sel:
                    continue
                if tname == "u":
                    src_t, bw = u_dup, 2 * Z
                elif tname == "pa":
                    src_t, bw = pa, 2 * Z
                else:
                    src_t, bw = work, Z
                a0 = (blk0 + (4 if tname == "pb" else 0)) * bw + off
                if nblk > 1:
                    dst = of[:, ds:ds + 4 * (Z * nblk - 1) + 1:4]
                    dst = dst.rearrange("p (a b) -> p a b", b=Z)
                    if bw == Z:
                        src = src_t[:, a0:a0 + (nblk - 1) * bw + Z]
                        src = src.rearrange("p (a b) -> p a b", b=bw)
                    else:
                        src = src_t.rearrange("p (a b) -> p a b", b=bw)[
                            :, blk0:blk0 + nblk, 0:Z]
                    ilv_op(dst, src, ln * nblk, eng)
                else:
                    dst = of[:, ds:ds + 4 * (ln - 1) + 1:4]
                    src = src_t[:, a0:a0 + ln]
                    ilv_op(dst, src, ln, eng)

        # u/pa-sourced spans only need input DMA / pa chain; emit at coarse
        # granularity (fewer, bigger copies)
        for c in range(4):
            emit_ilv(c, "u", nchunk=4)
        for c in range(4):
            emit_ilv(c, "pa", nchunk=4)

        # ---- remaining pb rows: one global wave pass (max pair-fusion),
        # rows ordered by first-needing chunk; emit each chunk's pb spans +
        # DMA as soon as every row it needs has completed ----
        needset = []
        for c in range(NCHUNK):
            s = set()
            for tname, blk0, off, nblk, ln, ds in _ilv_copies(c):
                if tname == "pb":
                    s.update(range(blk0, blk0 + nblk))
            needset.append(s)

        prio = []
        for c in range(NCHUNK):
            for r in sorted(needset[c]):
                if r not in prio and r not in dma_rows:
                    prio.append(r)
        for r in range(PB_BLOCKS):
            if r not in prio and r not in dma_rows:
                prio.append(r)

        rows_done = set(dma_rows)
        next_chunk = [0]

        def flush_chunks(force=False):
            while next_chunk[0] < NCHUNK:
                c = next_chunk[0]
                if not (force or needset[c] <= rows_done):
                    return
                # chunks left over after the final wave run their pb spans
                # on DVE itself: it has just finished the rows they need
                # and would otherwise idle
                emit_ilv(c, "pb", eng="dve" if force else None)
                nc.sync.dma_start(o_dram[:, c * cw:(c + 1) * cw],
                                  of[:, c * cw:(c + 1) * cw])
                next_chunk[0] += 1

        def on_done(key):
            rows_done.add(key[1])
            flush_chunks()

        pb_rows = [(("pb", r), pb_a(r), mk_state(gC1[r], gC2[r]))
                   for r in prio]
        run_waves(pb_rows, on_row_done=on_done)
        flush_chunks(force=True)

    return nc


def _get_program(a_rows, a_cols, bi_rows, bi_cols, c1_rows, c1_cols,
                 c2_rows, c2_cols):
    if "prog" in _CACHE:
        return _CACHE["prog"]
    entB = _base_entries(bi_rows, bi_cols)
    assert sorted(entB) == [(i, j, 0) for i in range(4) for j in range(i + 1)]
    gA = _group(_base_entries(a_rows, a_cols), 4, drop_bc=(21,))
    gC1 = _group(_base_entries(c1_rows, c1_cols), PB_BLOCKS, drop_bc=(21,))
    gC2 = _group(_base_entries(c2_rows, c2_cols), PB_BLOCKS)
    nc = _build_program(gA, gC1, gC2)
    nc.compile()
    _CACHE["prog"] = nc
    return nc


def kernel(u, a_rows, a_cols, bi_rows, bi_cols, c1_rows, c1_cols,
           c2_rows, c2_cols, out_int, **_ignored):
    from concourse.bass_utils import run_bass_kernel_spmd

    u = np.asarray(u)
    assert u.shape == (B_TOTAL, K)
    oi = np.asarray(out_int)
    expect = np.arange(N, dtype=oi.dtype).reshape(NBPS, NQ).T.ravel()
    assert np.array_equal(oi, expect), "unexpected output interleaver"

    nc = _get_program(a_rows, a_cols, bi_rows, bi_cols,
                      c1_rows, c1_cols, c2_rows, c2_cols)

    # host marshalling: pack 4 batch rows per uint16 lane (4-bit nibbles)
    ub = u.astype(np.uint16)
    in_maps = []
    for c in range(N_CORES):
        seg = ub[c * B_CORE:(c + 1) * B_CORE]
        packed = (seg[0 * P:1 * P] | (seg[1 * P:2 * P] << 4)
                  | (seg[2 * P:3 * P] << 8) | (seg[3 * P:4 * P] << 12))
        in_maps.append({"u": np.ascontiguousarray(packed)})

    res = run_bass_kernel_spmd(nc, in_maps, core_ids=list(range(N_CORES)))

    # unpack: nibble t of lane p = batch row t*64+p
    out = np.empty((B_TOTAL, N), np.float32)
    for c in range(N_CORES):
        oc = res.results[c]["out"]
        for t in range(PACK):
            rows = slice(c * B_CORE + t * P, c * B_CORE + (t + 1) * P)
            out[rows] = ((oc >> (4 * t)) & 1).astype(np.float32)
    return out


# revision 21
# speedup vs baseline: 1.0249x; 1.0249x over previous
"""5G LDPC BG1 encoder (k=8000, n=16000, r=0.5, Z=384) on 8 Trainium2 cores.

Strategy: data parallelism over the batch (2048 -> 8 cores x 256 rows) with
4-way nibble packing: 4 batch rows share one uint16 SBUF lane (row t*64+p ->
nibble t of partition p), so every engine op processes 4 codewords at once
and DMA moves 4x fewer bytes.  GF(2) addition is nibble-wise bitwise XOR
(DVE-only on TRN2); nibbles stay in {0,1} so the host recovers bits with a
shift-and-mask.  Circulant shifts use a halo copy of every 384-col block,
loaded by a second DMA pass straight from DRAM.  Independent XOR-chain steps
from two different rows are fused into one [P,2,384] DVE instruction via
hand-built access patterns (arbitrary stride between the two rows), halving
DVE instruction count.  The rate-matching interleaver (out[:,4j+i] =
c_short[i*4000+j]) runs as stride-4 packed copies split between Activation
and GpSimd: u/pa-sourced output phases are emitted early, pb-sourced spans
per chunk as parity rows complete.  Host work is layout-only: pack nibbles
in, shift-and-mask out.
"""
import numpy as np
from contextlib import ExitStack

Z = 384
KB = 22
K = 8000
N = 16000
K_LDPC = KB * Z          # 8448
NBPS = 4
NQ = N // NBPS           # 4000
PB_BLOCKS = 19           # only pb[0:7232] survives rate matching

B_TOTAL = 2048
N_CORES = 8
B_CORE = B_TOTAL // N_CORES   # 256
P = 64                        # partitions per core
PACK = 4                      # batch rows packed per uint16 lane (nibbles)
NCHUNK = 8                    # output column chunks of 2000

_CACHE = {}


def _base_entries(rows, cols):
    """Recover (base_row, base_col, shift) triplets from lifted index lists."""
    rows = np.asarray(rows, np.int64)
    cols = np.asarray(cols, np.int64)
    m = (rows % Z) == 0
    br = (rows[m] // Z).astype(int)
    bc = (cols[m] // Z).astype(int)
    sh = (cols[m] % Z).astype(int)
    return list(zip(br.tolist(), bc.tolist(), sh.tolist()))


def _group(entries, n_blocks, drop_bc=()):
    g = [[] for _ in range(n_blocks)]
    for br, bc, s in entries:
        if bc in drop_bc or br >= n_blocks:
            continue
        g[br].append((bc, s))
    return g


def _ilv_copies(chunk, nchunk=NCHUNK):
    """Interleaver copy specs for output chunk (cols [chunk*cw, +cw)):
    (tile, blk0, off, nblk, ln, dst_start_within_chunk).

    c_short = u_bits[768:8000] ++ pa[0:1536] ++ pb[0:7232], and
    out[:, 4j+i] = c_short[i*4000 + j]; chunk c covers j in
    [c*(NQ/nchunk), (c+1)*(NQ/nchunk)).
    """
    spans = ([("u", b, 0, Z) for b in range(2, 20)] + [("u", 20, 0, 320)]
             + [("pa", b, 0, Z) for b in range(4)]
             + [("pb", b, 0, Z) for b in range(18)] + [("pb", 18, 0, 320)])
    jlo, jhi = chunk * (NQ // nchunk), (chunk + 1) * (NQ // nchunk)
    out = []
    for i in range(NBPS):
        # phase i reads c_short[i*NQ + j] for j in [jlo, jhi); dst offsets
        # are absolute within the [P, N] output tile.
        glo, ghi = i * NQ + jlo, i * NQ + jhi
        g = 0
        pieces = []
        for tname, blk, off, ln in spans:
            a, b = max(g, glo), min(g + ln, ghi)
            if a < b:
                pieces.append((tname, blk, off + a - g, b - a,
                               4 * (a - glo) + i + 4 * jlo))
            g += ln
        merged = []
        for pc in pieces:
            tname, blk, off, ln, ds = pc
            if merged and off == 0 and ln == Z:
                mt, mb_, mo, mn, ml, mds = merged[-1]
                if mt == tname and mo == 0 and ml == Z and mb_ + mn == blk:
                    merged[-1] = (mt, mb_, mo, mn + 1, ml, mds)
                    continue
            merged.append((tname, blk, off, 1, ln, ds))
        out.extend(merged)
    return out


def _build_program(gA, gC1, gC2):
    import concourse.tile as tile
    from concourse import bacc, mybir
    from concourse.alu_op_type import AluOpType
    import bass_rust

    u16 = mybir.dt.uint16
    XOR = AluOpType.bitwise_xor
    VecI64Pair = bass_rust.VecI64Pair

    nc = bacc.Bacc("TRN2", target_bir_lowering=False, debug=False)
    u_dram = nc.dram_tensor("u", [P, K], u16, kind="ExternalInput").ap()
    o_dram = nc.dram_tensor("out", [P, N], u16, kind="ExternalOutput").ap()

    def pair_view(flat_ap, addr_a, addr_b, ln=Z):
        """[P, 2, ln] view of a flat [P, M] tile AP at two free offsets."""
        v = flat_ap[:, addr_a:addr_a + 1]
        w = v.copy()
        pstride = v.ap.to_list()[0]
        w.ap = VecI64Pair([pstride, [addr_b - addr_a, 2], [1, ln]])
        return w

    with tile.TileContext(nc) as tc, ExitStack() as ctx:
        pin = ctx.enter_context(tc.tile_pool(name="pin", bufs=1))
        pw = ctx.enter_context(tc.tile_pool(name="pw", bufs=1))
        pout = ctx.enter_context(tc.tile_pool(name="pout", bufs=1))

        # u_dup[p, bc*768 + 0:384] = info block bc; [.. 384:768] = halo copy
        # (blocks 0..20; block 21 is all-filler and dropped from the graph).
        # DMA in per block-group, main+halo interleaved, so XOR waves that
        # touch low blocks can start while later groups are still in flight.
        u_dup = pin.tile([P, 21 * 2 * Z], u16, tag="u_dup")
        u3 = u_dup.rearrange("p (a b) -> p a b", b=2 * Z)
        nc.gpsimd.memset(u3[:, 20, 320:Z], 0)
        nc.gpsimd.memset(u3[:, 20, Z + 320:2 * Z], 0)
        for lo, hi in ((0, 4), (4, 10), (10, 15), (15, 20)):
            src = u_dram[:, lo * Z:hi * Z].rearrange("p (a b) -> p a b", b=Z)
            nc.sync.dma_start(u3[:, lo:hi, 0:Z], src)
            nc.sync.dma_start(u3[:, lo:hi, Z:2 * Z], src)
        nc.sync.dma_start(u3[:, 20, 0:320], u_dram[:, 7680:8000])
        nc.sync.dma_start(u3[:, 20, Z:Z + 320], u_dram[:, 7680:8000])

        # work tile: au rows 0..3 then pb rows 0..18, each Z wide (flat)
        work = pw.tile([P, (4 + PB_BLOCKS) * Z], u16, tag="work")
        pa = pw.tile([P, 4 * 2 * Z], u16, tag="pa")
        pa3 = pa.rearrange("p (a b) -> p a b", b=2 * Z)

        def uaddr(bc, s):
            return bc * 2 * Z + s

        def paaddr(bc, s):
            return bc * 2 * Z + s

        def au_a(br):
            return br * Z

        def pb_a(r):
            return (4 + r) * Z

        # ---------- wave scheduler: pair-fuse independent chain steps ------
        # rows: au rows then pb rows.  Each row = (dst_addr, [steps]);
        # step = ('first', a1, a2) | ('accu', a) | ('accpa', a).
        emitted = {"n": 0}

        def emit_steps(s1, s2):
            """Emit one DVE instruction covering one or two chain steps."""
            if s2 is None:
                dst_a, st = s1
                if st[0] == "first":
                    nc.vector.tensor_tensor(
                        work[:, dst_a:dst_a + Z],
                        u_dup[:, st[1]:st[1] + Z],
                        u_dup[:, st[2]:st[2] + Z], op=XOR)
                else:
                    src = u_dup if st[0] == "accu" else pa
                    nc.vector.tensor_tensor(
                        work[:, dst_a:dst_a + Z],
                        work[:, dst_a:dst_a + Z],
                        src[:, st[1]:st[1] + Z], op=XOR)
                return
            (da, sta), (db, stb) = s1, s2
            dst = pair_view(work, da, db)
            if sta[0] == "first":
                nc.vector.tensor_tensor(
                    dst, pair_view(u_dup, sta[1], stb[1]),
                    pair_view(u_dup, sta[2], stb[2]), op=XOR)
            else:
                src = u_dup if sta[0] == "accu" else pa
                nc.vector.tensor_tensor(
                    dst, pair_view(work, da, db),
                    pair_view(src, sta[1], stb[1]), op=XOR)

        def run_waves(rows, on_row_done=None):
            """rows: list of (row_key, dst_addr, state) with state =
            {'first': (a1,a2)|None, 'accu': [a..], 'accpa': [a..]}.
            Two-server greedy by list order: each wave advances the foremost
            unfinished row, fused with the next row that can offer a
            same-class step ('first' must precede accs within a row; accu
            and accpa commute).  Completion order == list order."""
            def avail(st):
                if st["first"] is not None:
                    return ("first",)
                cl = []
                if st["accu"]:
                    cl.append("accu")
                if st["accpa"]:
                    cl.append("accpa")
                return tuple(cl)

            def take(st, cls):
                if cls == "first":
                    a1, a2 = st["first"]
                    st["first"] = None
                    return ("first", a1, a2)
                if cls == "accu":
                    return ("accu", st["accu"].pop(0))
                return ("accpa", st["accpa"].pop(0))

            def row_done(k):
                if not avail(rows[k][2]) and on_row_done:
                    on_row_done(rows[k][0])

            while True:
                act = [i for i, r in enumerate(rows) if avail(r[2])]
                if not act:
                    break
                i = act[0]
                ci = avail(rows[i][2])
                pick = None
                for j in act[1:]:
                    shared = [c for c in ci if c in avail(rows[j][2])]
                    if shared:
                        pick = (j, shared[0])
                        break
                if pick is None:
                    # prefer draining accu first (pa may not be ready early)
                    cls = ci[0]
                    emit_steps((rows[i][1], take(rows[i][2], cls)), None)
                    row_done(i)
                else:
                    j, cls = pick
                    emit_steps((rows[i][1], take(rows[i][2], cls)),
                               (rows[j][1], take(rows[j][2], cls)))
                    row_done(i)
                    row_done(j)

        def mk_state(c1_terms, c2_terms):
            us = sorted(uaddr(bc, s) for bc, s in c1_terms)
            pas = sorted(paaddr(bc, s) for bc, s in c2_terms)
            assert len(us) >= 2
            return {"first": (us[0], us[1]), "accu": us[2:], "accpa": pas}

        # ---- au = A @ u ----
        au_rows = [(("au", br), au_a(br), mk_state(gA[br], []))
                   for br in range(4)]
        run_waves(au_rows)

        # ---- pa = B_inv @ au = cumulative XOR, written to main AND halo
        # half in one dual-write op each (no separate halo copy) ----
        nc.vector.tensor_copy(pair_view(pa, 0, Z),
                              pair_view(work, au_a(0), au_a(0)))
        for i in range(1, 4):
            nc.vector.tensor_tensor(
                pair_view(pa, i * 2 * Z, i * 2 * Z + Z),
                pair_view(pa, (i - 1) * 2 * Z, (i - 1) * 2 * Z),
                pair_view(work, au_a(i), au_a(i)), op=XOR)

        # ---- pb rows 16..18: integer add-accumulate chains on the software
        # DGE (nibble counts <= 15; host parity-extracts).  Costs ~1us of
        # GpSimd time per hop but runs off the DVE critical path.  WAW on
        # the destination serializes each chain; u-hops first so the queue
        # never stalls waiting for pa. ----
        dma_rows = tuple(r for r in (16, 17, 18)
                         if len(gC1[r]) + len(gC2[r]) <= 15)
        hops_u, hops_pa = [], []
        for r in dma_rows:
            us = sorted(uaddr(bc, s) for bc, s in gC1[r])
            pas = sorted(paaddr(bc, s) for bc, s in gC2[r])
            hops_u.append((r, us))
            hops_pa.append((r, pas))
        for r, us in hops_u:
            dst = work[:, pb_a(r):pb_a(r) + Z]
            nc.gpsimd.dma_start(dst, u_dup[:, us[0]:us[0] + Z])
            for a in us[1:]:
                nc.gpsimd.dma_start(dst, u_dup[:, a:a + Z],
                                    accum_op=AluOpType.add)
        for r, pas in hops_pa:
            dst = work[:, pb_a(r):pb_a(r) + Z]
            for a in pas:
                nc.gpsimd.dma_start(dst, pa[:, a:a + Z],
                                    accum_op=AluOpType.add)

        # ---- interleave: balance Act vs Pool by cost (Pool pre-loaded
        # with the SWDGE hop cost above) ----
        of = pout.tile([P, N], u16, tag="of")
        cw = N // NCHUNK
        nhops = sum(len(us) for _, us in hops_u) + sum(
            len(p) for _, p in hops_pa)
        ebusy = {"act": 0.0, "pool": 1020.0 * nhops}

        def ilv_op(dst, src, ln, eng=None):
            if eng == "dve":
                nc.vector.tensor_copy(dst, src)
                return
            ca = 0.833 * ln + 185          # Activation: fast, big fixed cost
            cp = 1.389 * ln + 60           # GpSimd: slower, no fixed cost
            if ebusy["act"] + ca <= ebusy["pool"] + cp:
                ebusy["act"] += ca
                nc.scalar.copy(dst, src)
            else:
                ebusy["pool"] += cp
                nc.gpsimd.tensor_copy(dst, src)

        def emit_ilv(c, tname_sel, nchunk=NCHUNK, eng=None):
            for tname, blk0, off, nblk, ln, ds in _ilv_copies(c, nchunk):
                if tname != tname_sel:
                    continue
                if tname == "u":
                    src_t, bw = u_dup, 2 * Z
                elif tname == "pa":
                    src_t, bw = pa, 2 * Z
                else:
                    src_t, bw = work, Z
                a0 = (blk0 + (4 if tname == "pb" else 0)) * bw + off
                if nblk > 1:
                    dst = of[:, ds:ds + 4 * (Z * nblk - 1) + 1:4]
                    dst = dst.rearrange("p (a b) -> p a b", b=Z)
                    if bw == Z:
                        src = src_t[:, a0:a0 + (nblk - 1) * bw + Z]
                        src = src.rearrange("p (a b) -> p a b", b=bw)
                    else:
                        src = src_t.rearrange("p (a b) -> p a b", b=bw)[
                            :, blk0:blk0 + nblk, 0:Z]
                    ilv_op(dst, src, ln * nblk, eng)
                else:
                    dst = of[:, ds:ds + 4 * (ln - 1) + 1:4]
                    src = src_t[:, a0:a0 + ln]
                    ilv_op(dst, src, ln, eng)

        # u/pa-sourced spans only need input DMA / pa chain; emit at coarse
        # granularity (fewer, bigger copies)
        for c in range(4):
            emit_ilv(c, "u", nchunk=4)
        for c in range(4):
            emit_ilv(c, "pa", nchunk=4)

        # ---- remaining pb rows: one global wave pass (max pair-fusion),
        # rows ordered by first-needing chunk; emit each chunk's pb spans +
        # DMA as soon as every row it needs has completed ----
        needset = []
        for c in range(NCHUNK):
            s = set()
            for tname, blk0, off, nblk, ln, ds in _ilv_copies(c):
                if tname == "pb":
                    s.update(range(blk0, blk0 + nblk))
            needset.append(s)

        prio = []
        for c in range(NCHUNK):
            for r in sorted(needset[c]):
                if r not in prio and r not in dma_rows:
                    prio.append(r)
        for r in range(PB_BLOCKS):
            if r not in prio and r not in dma_rows:
                prio.append(r)

        rows_done = set(dma_rows)
        next_chunk = [0]

        def flush_chunks(force=False):
            while next_chunk[0] < NCHUNK:
                c = next_chunk[0]
                if not (force or needset[c] <= rows_done):
                    return
                # chunks left over after the final wave run their pb spans
                # on DVE itself: it has just finished the rows they need
                # and would otherwise idle
                emit_ilv(c, "pb", eng="dve" if force else None)
                nc.sync.dma_start(o_dram[:, c * cw:(c + 1) * cw],
                                  of[:, c * cw:(c + 1) * cw])
                next_chunk[0] += 1

        def on_done(key):
            rows_done.add(key[1])
            flush_chunks()

        pb_rows = [(("pb", r), pb_a(r), mk_state(gC1[r], gC2[r]))
                   for r in prio]
        run_waves(pb_rows, on_row_done=on_done)
        flush_chunks(force=True)

    return nc


def _get_program(a_rows, a_cols, bi_rows, bi_cols, c1_rows, c1_cols,
                 c2_rows, c2_cols):
    if "prog" in _CACHE:
        return _CACHE["prog"]
    entB = _base_entries(bi_rows, bi_cols)
    assert sorted(entB) == [(i, j, 0) for i in range(4) for j in range(i + 1)]
    gA = _group(_base_entries(a_rows, a_cols), 4, drop_bc=(21,))
    gC1 = _group(_base_entries(c1_rows, c1_cols), PB_BLOCKS, drop_bc=(21,))
    gC2 = _group(_base_entries(c2_rows, c2_cols), PB_BLOCKS)
    nc = _build_program(gA, gC1, gC2)
    nc.compile()
    _CACHE["prog"] = nc
    return nc


def kernel(u, a_rows, a_cols, bi_rows, bi_cols, c1_rows, c1_cols,
           c2_rows, c2_cols, out_int, **_ignored):
    from concourse.bass_utils import run_bass_kernel_spmd

    u = np.asarray(u)
    assert u.shape == (B_TOTAL, K)
    oi = np.asarray(out_int)
    expect = np.arange(N, dtype=oi.dtype).reshape(NBPS, NQ).T.ravel()
    assert np.array_equal(oi, expect), "unexpected output interleaver"

    nc = _get_program(a_rows, a_cols, bi_rows, bi_cols,
                      c1_rows, c1_cols, c2_rows, c2_cols)

    # host marshalling: pack 4 batch rows per uint16 lane (4-bit nibbles)
    ub = u.astype(np.uint16)
    in_maps = []
    for c in range(N_CORES):
        seg = ub[c * B_CORE:(c + 1) * B_CORE]
        packed = (seg[0 * P:1 * P] | (seg[1 * P:2 * P] << 4)
                  | (seg[2 * P:3 * P] << 8) | (seg[3 * P:4 * P] << 12))
        in_maps.append({"u": np.ascontiguousarray(packed)})

    res = run_bass_kernel_spmd(nc, in_maps, core_ids=list(range(N_CORES)))

    # unpack: nibble t of lane p = batch row t*64+p
    out = np.empty((B_TOTAL, N), np.float32)
    for c in range(N_CORES):
        oc = res.results[c]["out"]
        for t in range(PACK):
            rows = slice(c * B_CORE + t * P, c * B_CORE + (t + 1) * P)
            out[rows] = ((oc >> (4 * t)) & 1).astype(np.float32)
    return out


# revision 22
# speedup vs baseline: 1.0995x; 1.0727x over previous
"""5G LDPC BG1 encoder (k=8000, n=16000, r=0.5, Z=384) on 8 Trainium2 cores.

Strategy: data parallelism over the batch (2048 -> 8 cores x 256 rows) with
4-way nibble packing: 4 batch rows share one uint16 SBUF lane (row t*64+p ->
nibble t of partition p), so every engine op processes 4 codewords at once
and DMA moves 4x fewer bytes.  GF(2) addition is nibble-wise bitwise XOR
(DVE-only on TRN2); nibbles stay in {0,1} so the host recovers bits with a
shift-and-mask.  Circulant shifts use a halo copy of every 384-col block,
loaded by a second DMA pass straight from DRAM.  Independent XOR-chain steps
from two different rows are fused into one [P,2,384] DVE instruction via
hand-built access patterns (arbitrary stride between the two rows), halving
DVE instruction count.  The rate-matching interleaver (out[:,4j+i] =
c_short[i*4000+j]) runs as stride-4 packed copies split between Activation
and GpSimd: u/pa-sourced output phases are emitted early, pb-sourced spans
per chunk as parity rows complete.  Host work is layout-only: pack nibbles
in, shift-and-mask out.
"""
import numpy as np
from contextlib import ExitStack

Z = 384
KB = 22
K = 8000
N = 16000
K_LDPC = KB * Z          # 8448
NBPS = 4
NQ = N // NBPS           # 4000
PB_BLOCKS = 19           # only pb[0:7232] survives rate matching

B_TOTAL = 2048
N_CORES = 8
B_CORE = B_TOTAL // N_CORES   # 256
P = 64                        # partitions per core
PACK = 4                      # batch rows packed per uint16 lane (nibbles)
NCHUNK = 8                    # output column chunks of 2000

_CACHE = {}


def _base_entries(rows, cols):
    """Recover (base_row, base_col, shift) triplets from lifted index lists."""
    rows = np.asarray(rows, np.int64)
    cols = np.asarray(cols, np.int64)
    m = (rows % Z) == 0
    br = (rows[m] // Z).astype(int)
    bc = (cols[m] // Z).astype(int)
    sh = (cols[m] % Z).astype(int)
    return list(zip(br.tolist(), bc.tolist(), sh.tolist()))


def _group(entries, n_blocks, drop_bc=()):
    g = [[] for _ in range(n_blocks)]
    for br, bc, s in entries:
        if bc in drop_bc or br >= n_blocks:
            continue
        g[br].append((bc, s))
    return g


def _ilv_copies(chunk, nchunk=NCHUNK):
    """Interleaver copy specs for output chunk (cols [chunk*cw, +cw)):
    (tile, blk0, off, nblk, ln, dst_start_within_chunk).

    c_short = u_bits[768:8000] ++ pa[0:1536] ++ pb[0:7232], and
    out[:, 4j+i] = c_short[i*4000 + j]; chunk c covers j in
    [c*(NQ/nchunk), (c+1)*(NQ/nchunk)).
    """
    spans = ([("u", b, 0, Z) for b in range(2, 20)] + [("u", 20, 0, 320)]
             + [("pa", b, 0, Z) for b in range(4)]
             + [("pb", b, 0, Z) for b in range(18)] + [("pb", 18, 0, 320)])
    jlo, jhi = chunk * (NQ // nchunk), (chunk + 1) * (NQ // nchunk)
    out = []
    for i in range(NBPS):
        # phase i reads c_short[i*NQ + j] for j in [jlo, jhi); dst offsets
        # are absolute within the [P, N] output tile.
        glo, ghi = i * NQ + jlo, i * NQ + jhi
        g = 0
        pieces = []
        for tname, blk, off, ln in spans:
            a, b = max(g, glo), min(g + ln, ghi)
            if a < b:
                pieces.append((tname, blk, off + a - g, b - a,
                               4 * (a - glo) + i + 4 * jlo))
            g += ln
        merged = []
        for pc in pieces:
            tname, blk, off, ln, ds = pc
            if merged and off == 0 and ln == Z:
                mt, mb_, mo, mn, ml, mds = merged[-1]
                if mt == tname and mo == 0 and ml == Z and mb_ + mn == blk:
                    merged[-1] = (mt, mb_, mo, mn + 1, ml, mds)
                    continue
            merged.append((tname, blk, off, 1, ln, ds))
        out.extend(merged)
    return out


def _build_program(gA, gC1, gC2):
    import concourse.tile as tile
    from concourse import bacc, mybir
    from concourse.alu_op_type import AluOpType
    import bass_rust

    u16 = mybir.dt.uint16
    XOR = AluOpType.bitwise_xor
    VecI64Pair = bass_rust.VecI64Pair

    nc = bacc.Bacc("TRN2", target_bir_lowering=False, debug=False)
    u_dram = nc.dram_tensor("u", [P, K], u16, kind="ExternalInput").ap()
    o_dram = nc.dram_tensor("out", [P, N], u16, kind="ExternalOutput").ap()

    def pair_view(flat_ap, addr_a, addr_b, ln=Z):
        """[P, 2, ln] view of a flat [P, M] tile AP at two free offsets."""
        v = flat_ap[:, addr_a:addr_a + 1]
        w = v.copy()
        pstride = v.ap.to_list()[0]
        w.ap = VecI64Pair([pstride, [addr_b - addr_a, 2], [1, ln]])
        return w

    with tile.TileContext(nc) as tc, ExitStack() as ctx:
        pin = ctx.enter_context(tc.tile_pool(name="pin", bufs=1))
        pw = ctx.enter_context(tc.tile_pool(name="pw", bufs=1))
        pout = ctx.enter_context(tc.tile_pool(name="pout", bufs=1))

        # u_dup[p, bc*768 + 0:384] = info block bc; [.. 384:768] = halo copy
        # (blocks 0..20; block 21 is all-filler and dropped from the graph).
        # DMA in per block-group, main+halo interleaved, so XOR waves that
        # touch low blocks can start while later groups are still in flight.
        u_dup = pin.tile([P, 21 * 2 * Z], u16, tag="u_dup")
        u3 = u_dup.rearrange("p (a b) -> p a b", b=2 * Z)
        nc.gpsimd.memset(u3[:, 20, 320:Z], 0)
        nc.gpsimd.memset(u3[:, 20, Z + 320:2 * Z], 0)
        for lo, hi in ((0, 4), (4, 10), (10, 15), (15, 20)):
            src = u_dram[:, lo * Z:hi * Z].rearrange("p (a b) -> p a b", b=Z)
            nc.sync.dma_start(u3[:, lo:hi, 0:Z], src)
            nc.sync.dma_start(u3[:, lo:hi, Z:2 * Z], src)
        nc.sync.dma_start(u3[:, 20, 0:320], u_dram[:, 7680:8000])
        nc.sync.dma_start(u3[:, 20, Z:Z + 320], u_dram[:, 7680:8000])

        # work tile: au rows 0..3 then pb rows 0..18, each Z wide (flat)
        work = pw.tile([P, (4 + PB_BLOCKS) * Z], u16, tag="work")
        pa = pw.tile([P, 4 * 2 * Z], u16, tag="pa")
        pa3 = pa.rearrange("p (a b) -> p a b", b=2 * Z)

        def uaddr(bc, s):
            return bc * 2 * Z + s

        def paaddr(bc, s):
            return bc * 2 * Z + s

        def au_a(br):
            return br * Z

        def pb_a(r):
            return (4 + r) * Z

        # ---------- wave scheduler: pair-fuse independent chain steps ------
        # rows: au rows then pb rows.  Each row = (dst_addr, [steps]);
        # step = ('first', a1, a2) | ('accu', a) | ('accpa', a).
        emitted = {"n": 0}

        def emit_steps(s1, s2):
            """Emit one DVE instruction covering one or two chain steps."""
            if s2 is None:
                dst_a, st = s1
                if st[0] == "first":
                    nc.vector.tensor_tensor(
                        work[:, dst_a:dst_a + Z],
                        u_dup[:, st[1]:st[1] + Z],
                        u_dup[:, st[2]:st[2] + Z], op=XOR)
                else:
                    src = u_dup if st[0] == "accu" else pa
                    nc.vector.tensor_tensor(
                        work[:, dst_a:dst_a + Z],
                        work[:, dst_a:dst_a + Z],
                        src[:, st[1]:st[1] + Z], op=XOR)
                return
            (da, sta), (db, stb) = s1, s2
            dst = pair_view(work, da, db)
            if sta[0] == "first":
                nc.vector.tensor_tensor(
                    dst, pair_view(u_dup, sta[1], stb[1]),
                    pair_view(u_dup, sta[2], stb[2]), op=XOR)
            else:
                src = u_dup if sta[0] == "accu" else pa
                nc.vector.tensor_tensor(
                    dst, pair_view(work, da, db),
                    pair_view(src, sta[1], stb[1]), op=XOR)

        def run_waves(rows, on_row_done=None):
            """rows: list of (row_key, dst_addr, state) with state =
            {'first': (a1,a2)|None, 'accu': [a..], 'accpa': [a..]}.
            Two-server greedy by list order: each wave advances the foremost
            unfinished row, fused with the next row that can offer a
            same-class step ('first' must precede accs within a row; accu
            and accpa commute).  Completion order == list order."""
            def avail(st):
                if st["first"] is not None:
                    return ("first",)
                cl = []
                if st["accu"]:
                    cl.append("accu")
                if st["accpa"]:
                    cl.append("accpa")
                return tuple(cl)

            def take(st, cls):
                if cls == "first":
                    a1, a2 = st["first"]
                    st["first"] = None
                    return ("first", a1, a2)
                if cls == "accu":
                    return ("accu", st["accu"].pop(0))
                return ("accpa", st["accpa"].pop(0))

            def row_done(k):
                if not avail(rows[k][2]) and on_row_done:
                    on_row_done(rows[k][0])

            while True:
                act = [i for i, r in enumerate(rows) if avail(r[2])]
                if not act:
                    break
                i = act[0]
                ci = avail(rows[i][2])
                pick = None
                for j in act[1:]:
                    shared = [c for c in ci if c in avail(rows[j][2])]
                    if shared:
                        pick = (j, shared[0])
                        break
                if pick is None:
                    # prefer draining accu first (pa may not be ready early)
                    cls = ci[0]
                    emit_steps((rows[i][1], take(rows[i][2], cls)), None)
                    row_done(i)
                else:
                    j, cls = pick
                    emit_steps((rows[i][1], take(rows[i][2], cls)),
                               (rows[j][1], take(rows[j][2], cls)))
                    row_done(i)
                    row_done(j)

        def mk_state(c1_terms, c2_terms):
            us = sorted(uaddr(bc, s) for bc, s in c1_terms)
            pas = sorted(paaddr(bc, s) for bc, s in c2_terms)
            assert len(us) >= 2
            return {"first": (us[0], us[1]), "accu": us[2:], "accpa": pas}

        # ---- au = A @ u ----
        au_rows = [(("au", br), au_a(br), mk_state(gA[br], []))
                   for br in range(4)]
        run_waves(au_rows)

        # ---- pa = B_inv @ au = cumulative XOR, written to main AND halo
        # half in one dual-write op each (no separate halo copy) ----
        nc.vector.tensor_copy(pair_view(pa, 0, Z),
                              pair_view(work, au_a(0), au_a(0)))
        for i in range(1, 4):
            nc.vector.tensor_tensor(
                pair_view(pa, i * 2 * Z, i * 2 * Z + Z),
                pair_view(pa, (i - 1) * 2 * Z, (i - 1) * 2 * Z),
                pair_view(work, au_a(i), au_a(i)), op=XOR)

        # ---- pb rows 16..18: integer add-accumulate chains on the software
        # DGE (nibble counts <= 15; host parity-extracts).  Costs ~1us of
        # GpSimd time per hop but runs off the DVE critical path.  WAW on
        # the destination serializes each chain; u-hops first so the queue
        # never stalls waiting for pa. ----
        dma_rows = tuple(r for r in (16, 17, 18)
                         if len(gC1[r]) + len(gC2[r]) <= 15)
        hops_u, hops_pa = [], []
        for r in dma_rows:
            us = sorted(uaddr(bc, s) for bc, s in gC1[r])
            pas = sorted(paaddr(bc, s) for bc, s in gC2[r])
            hops_u.append((r, us))
            hops_pa.append((r, pas))
        for r, us in hops_u:
            dst = work[:, pb_a(r):pb_a(r) + Z]
            nc.gpsimd.dma_start(dst, u_dup[:, us[0]:us[0] + Z])
            for a in us[1:]:
                nc.gpsimd.dma_start(dst, u_dup[:, a:a + Z],
                                    accum_op=AluOpType.add)
        for r, pas in hops_pa:
            dst = work[:, pb_a(r):pb_a(r) + Z]
            for a in pas:
                nc.gpsimd.dma_start(dst, pa[:, a:a + Z],
                                    accum_op=AluOpType.add)

        # ---- interleave: balance Act vs Pool by cost (Pool pre-loaded
        # with the SWDGE hop cost above) ----
        of = pout.tile([P, N], u16, tag="of")
        cw = N // NCHUNK
        nhops = sum(len(us) for _, us in hops_u) + sum(
            len(p) for _, p in hops_pa)
        ebusy = {"act": 0.0, "pool": 1020.0 * nhops}

        def ilv_op(dst, src, ln, eng=None):
            if eng == "dve":
                nc.vector.tensor_copy(dst, src)
                return
            ca = 0.833 * ln + 370          # Activation: fast, big fixed cost
            cp = 1.389 * ln + 60           # GpSimd: slower, no fixed cost
            if ebusy["act"] + ca <= ebusy["pool"] + cp:
                ebusy["act"] += ca
                nc.scalar.copy(dst, src)
            else:
                ebusy["pool"] += cp
                nc.gpsimd.tensor_copy(dst, src)

        def emit_ilv(c, tname_sel, nchunk=NCHUNK, eng=None):
            for tname, blk0, off, nblk, ln, ds in _ilv_copies(c, nchunk):
                if tname != tname_sel:
                    continue
                if tname == "u":
                    src_t, bw = u_dup, 2 * Z
                elif tname == "pa":
                    src_t, bw = pa, 2 * Z
                else:
                    src_t, bw = work, Z
                a0 = (blk0 + (4 if tname == "pb" else 0)) * bw + off
                if nblk > 1:
                    dst = of[:, ds:ds + 4 * (Z * nblk - 1) + 1:4]
                    dst = dst.rearrange("p (a b) -> p a b", b=Z)
                    if bw == Z:
                        src = src_t[:, a0:a0 + (nblk - 1) * bw + Z]
                        src = src.rearrange("p (a b) -> p a b", b=bw)
                    else:
                        src = src_t.rearrange("p (a b) -> p a b", b=bw)[
                            :, blk0:blk0 + nblk, 0:Z]
                    ilv_op(dst, src, ln * nblk, eng)
                else:
                    dst = of[:, ds:ds + 4 * (ln - 1) + 1:4]
                    src = src_t[:, a0:a0 + ln]
                    ilv_op(dst, src, ln, eng)

        # u/pa-sourced spans only need input DMA / pa chain; emit at coarse
        # granularity (fewer, bigger copies)
        for c in range(4):
            emit_ilv(c, "u", nchunk=4)
        for c in range(4):
            emit_ilv(c, "pa", nchunk=4)

        # ---- remaining pb rows: one global wave pass (max pair-fusion),
        # rows ordered by first-needing chunk; emit each chunk's pb spans +
        # DMA as soon as every row it needs has completed ----
        needset = []
        for c in range(NCHUNK):
            s = set()
            for tname, blk0, off, nblk, ln, ds in _ilv_copies(c):
                if tname == "pb":
                    s.update(range(blk0, blk0 + nblk))
            needset.append(s)

        prio = []
        for c in range(NCHUNK):
            for r in sorted(needset[c]):
                if r not in prio and r not in dma_rows:
                    prio.append(r)
        for r in range(PB_BLOCKS):
            if r not in prio and r not in dma_rows:
                prio.append(r)

        rows_done = set(dma_rows)
        next_chunk = [0]

        def flush_chunks(force=False):
            while next_chunk[0] < NCHUNK:
                c = next_chunk[0]
                if not (force or needset[c] <= rows_done):
                    return
                # chunks left over after the final wave run their pb spans
                # on DVE itself: it has just finished the rows they need
                # and would otherwise idle
                emit_ilv(c, "pb", eng="dve" if force else None)
                nc.sync.dma_start(o_dram[:, c * cw:(c + 1) * cw],
                                  of[:, c * cw:(c + 1) * cw])
                next_chunk[0] += 1

        def on_done(key):
            rows_done.add(key[1])
            flush_chunks()

        pb_rows = [(("pb", r), pb_a(r), mk_state(gC1[r], gC2[r]))
                   for r in prio]
        run_waves(pb_rows, on_row_done=on_done)
        flush_chunks(force=True)

    return nc


def _get_program(a_rows, a_cols, bi_rows, bi_cols, c1_rows, c1_cols,
                 c2_rows, c2_cols):
    if "prog" in _CACHE:
        return _CACHE["prog"]
    entB = _base_entries(bi_rows, bi_cols)
    assert sorted(entB) == [(i, j, 0) for i in range(4) for j in range(i + 1)]
    gA = _group(_base_entries(a_rows, a_cols), 4, drop_bc=(21,))
    gC1 = _group(_base_entries(c1_rows, c1_cols), PB_BLOCKS, drop_bc=(21,))
    gC2 = _group(_base_entries(c2_rows, c2_cols), PB_BLOCKS)
    nc = _build_program(gA, gC1, gC2)
    nc.compile()
    _CACHE["prog"] = nc
    return nc


def kernel(u, a_rows, a_cols, bi_rows, bi_cols, c1_rows, c1_cols,
           c2_rows, c2_cols, out_int, **_ignored):
    from concourse.bass_utils import run_bass_kernel_spmd

    u = np.asarray(u)
    assert u.shape == (B_TOTAL, K)
    oi = np.asarray(out_int)
    expect = np.arange(N, dtype=oi.dtype).reshape(NBPS, NQ).T.ravel()
    assert np.array_equal(oi, expect), "unexpected output interleaver"

    nc = _get_program(a_rows, a_cols, bi_rows, bi_cols,
                      c1_rows, c1_cols, c2_rows, c2_cols)

    # host marshalling: pack 4 batch rows per uint16 lane (4-bit nibbles)
    ub = u.astype(np.uint16)
    in_maps = []
    for c in range(N_CORES):
        seg = ub[c * B_CORE:(c + 1) * B_CORE]
        packed = (seg[0 * P:1 * P] | (seg[1 * P:2 * P] << 4)
                  | (seg[2 * P:3 * P] << 8) | (seg[3 * P:4 * P] << 12))
        in_maps.append({"u": np.ascontiguousarray(packed)})

    res = run_bass_kernel_spmd(nc, in_maps, core_ids=list(range(N_CORES)))

    # unpack: nibble t of lane p = batch row t*64+p
    out = np.empty((B_TOTAL, N), np.float32)
    for c in range(N_CORES):
        oc = res.results[c]["out"]
        for t in range(PACK):
            rows = slice(c * B_CORE + t * P, c * B_CORE + (t + 1) * P)
            out[rows] = ((oc >> (4 * t)) & 1).astype(np.float32)
    return out
